# revision 24
# baseline (speedup 1.0000x reference)
"""Conv2D 3x3 (NCHW, OIHW, stride 1, pad 1) on 8 Trainium2 NeuronCores.

Problem shape: input (32, 128, 56, 56) fp32, weights (256, 128, 3, 3) fp32,
output (32, 256, 56, 56) fp32.

Strategy (v2 — 1D Winograd F(2,3) along the width axis):
  - Data-parallel over batch: 4 images per core, weights replicated.
  - Host precomputes the Winograd input transform: for each padded row and
    2-wide output tile t, the 4 transform planes
        v0 = d[2t]   - d[2t+2]
        v1 = d[2t+1] + d[2t+2]
        v2 = d[2t+2] - d[2t+1]
        v3 = d[2t+1] - d[2t+3]
    giving V[ci, k, 58 rows, 28 tiles] (fp16). Weights become
    U[dy,k][ci,co] = rows of G @ g (fp16).
  - Device: per (image, co-half, 16-row chunk) accumulate in PSUM
        m_k = sum_dy U[dy,k].T @ V[k][rows+dy]     (4 planes, 12 matmuls)
    with moving dim 448 = 16 rows x 28 tiles. This is 2/3 of the direct
    conv's MAC count (12x448 vs 9x(2x448) per 16 output rows).
  - Output transform split across engines so PE stays the bottleneck:
        ACT:  C1 = m1 (copy to SBUF fp16), C2 = m2
        DVE:  A = m0 + C1, B = C1 - C2 (2x fp16), y_even = A + C2 (2x),
              y_odd = B - m3
    Even/odd output columns are stored as separate planes; host interleaves.
"""

import sys

sys.path.insert(0, "/opt/trn_rl_repo")

import numpy as np

N_CORES = 8
N_FULL = 32
IMGS = N_FULL // N_CORES  # images per core
CIN = 128
COUT = 256
H = W = 56
HP = 58  # padded rows
T = 28  # winograd tiles per row
PLANE = HP * T  # 1624 elements per transform plane
NK = 4  # transform planes
OUTP = H * T  # 1568 output elements per phase (even/odd)

# row chunks per (image, half): 16+16+16+8 output rows
CHUNK_R0 = (0, 16, 32, 48)
CHUNK_H = (16, 16, 16, 8)

_CACHE = {}


def _split_sync_waits(nc, mybir, max_waits=1):
    """The walrus build in this container rejects instructions carrying
    more than one semaphore wait; hoist extras onto preceding NOPs on the
    same engine (engine executes them in order, semantics preserved)."""
    ctr = 0
    for f in nc.m.functions:
        for bb in f.blocks:
            new_insts = []
            for ins in bb.instructions:
                si = getattr(ins, "sync_info", None)
                if si is not None and si.on_wait and len(si.on_wait) > max_waits:
                    waits = list(si.on_wait)
                    extra, keep = waits[:-max_waits], waits[-max_waits:]
                    for i in range(0, len(extra), max_waits):
                        ctr += 1
                        nop = mybir.InstNoOp(
                            name=f"{ins.name}_wsplit{ctr}",
                            engine=ins.engine,
                            sync_info=mybir.SyncInfo(
                                on_wait=extra[i : i + max_waits], on_update=[]
                            ),
                            bass_nofuse=True,
                        )
                        new_insts.append(nop)
                    si.on_wait = keep
                new_insts.append(ins)
            bb.instructions[:] = new_insts
    return ctr


def _build():
    import concourse.bass as bass
    import concourse.mybir as mybir
    import concourse.tile as tile

    f32 = mybir.dt.float32
    f16 = mybir.dt.float16
    ADD = mybir.AluOpType.add
    SUB = mybir.AluOpType.subtract

    nc = bass.Bass()
    x = nc.declare_dram_parameter("x", [IMGS, CIN, NK * PLANE], f16, isOutput=False)
    w = nc.declare_dram_parameter("w", [CIN, 2 * NK * 3 * 128], f16, isOutput=False)
    # out[n, half, co, chunk*2+phase, 448 (padded)]; phase 0 = even cols
    out = nc.declare_dram_parameter("out", [IMGS, 2, 128, 8 * 448], f16, isOutput=True)

    x3 = x.rearrange("n p (k e) -> n p k e", k=NK)

    with tile.TileContext(nc) as tc:
        with (
            tc.tile_pool(name="wpool", bufs=1) as wpool,
            tc.tile_pool(name="vpool", bufs=2) as vpool,
            tc.tile_pool(name="tpool", bufs=3) as tpool,
            tc.tile_pool(name="opool", bufs=4) as opool,
            tc.tile_pool(name="psum", bufs=8, space="PSUM") as pspool,
        ):
            # Short PE warmup (~1.7us) bridges the gap until the first
            # input DMA lands; the first real matmuls then run inside the
            # remaining HAM cold window (1.2 GHz until ~3.4us of PE busy).
            warm = wpool.tile([128, 256], f16, name="warm")
            nc.vector.memzero(warm[:])
            wps = pspool.tile([128, 448], f32, name="ps")
            for _ in range(10):
                nc.tensor.matmul(
                    wps[:, 0:256], lhsT=warm[:, 0:128], rhs=warm[:], start=True, stop=True
                )

            # wt pieces are interleaved with image-0's input pieces below so
            # each lands just before its first use: h0/k0 weights first, the
            # rest of h0 on the sync ring behind the k0/k2 input heads, h1
            # weights later on the scalar ring.
            wt = wpool.tile([CIN, 2 * NK * 3 * 128], f16)
            nc.scalar.dma_start(out=wt[:, 0:384], in_=w[:, 0:384])

            def uslice(h, k, dy):
                c0 = ((h * NK + k) * 3 + dy) * 128
                return wt[:, c0 : c0 + 128]

            def emit_v_dmas(n, vt3):
                if n == 0:
                    # fine-grained pieces: plane k's head rows (18 rows,
                    # 129KB) land just before that plane's first (cold)
                    # matmuls. Alternate rings so transfers run on disjoint
                    # DMA queue sets in parallel.
                    # interleave input-plane heads with the weight pieces in
                    # first-use order across both rings (boot-time DMA
                    # bandwidth is scarce; every byte before its consumer)
                    nc.sync.dma_start(out=vt3[:, 0, 0 : 18 * T], in_=x3[n, :, 0, 0 : 18 * T])
                    nc.scalar.dma_start(out=vt3[:, 1, 0 : 18 * T], in_=x3[n, :, 1, 0 : 18 * T])
                    nc.sync.dma_start(out=wt[:, 384:768], in_=w[:, 384:768])
                    nc.sync.dma_start(out=wt[:, 768:1152], in_=w[:, 768:1152])
                    nc.scalar.dma_start(out=vt3[:, 2, 0 : 18 * T], in_=x3[n, :, 2, 0 : 18 * T])
                    nc.sync.dma_start(out=vt3[:, 3, 0 : 18 * T], in_=x3[n, :, 3, 0 : 18 * T])
                    nc.scalar.dma_start(out=wt[:, 1152:1536], in_=w[:, 1152:1536])
                    nc.sync.dma_start(out=wt[:, 1536:1920], in_=w[:, 1536:1920])
                    nc.scalar.dma_start(out=wt[:, 1920:3072], in_=w[:, 1920:3072])
                    nc.sync.dma_start(
                        out=vt3[:, :, 18 * T : 34 * T], in_=x3[n, :, :, 18 * T : 34 * T]
                    )
                    nc.scalar.dma_start(
                        out=vt3[:, :, 34 * T : 50 * T], in_=x3[n, :, :, 34 * T : 50 * T]
                    )
                    nc.scalar.dma_start(
                        out=vt3[:, :, 50 * T : PLANE], in_=x3[n, :, :, 50 * T : PLANE]
                    )
                else:
                    # later images prefetch on the scalar ring: the sync
                    # ring stays dedicated to out-DMAs
                    nc.scalar.dma_start(
                        out=vt3[:, :, 0 : 18 * T], in_=x3[n, :, :, 0 : 18 * T]
                    )
                    nc.scalar.dma_start(
                        out=vt3[:, :, 18 * T : 35 * T], in_=x3[n, :, :, 18 * T : 35 * T]
                    )
                    nc.scalar.dma_start(
                        out=vt3[:, :, 35 * T : PLANE], in_=x3[n, :, :, 35 * T : PLANE]
                    )

            vt = vpool.tile([CIN, NK * PLANE], f16)
            vt3 = vt.rearrange("p (k e) -> p k e", k=NK)
            emit_v_dmas(0, vt3)

            for n in range(IMGS):
                # chunk-major over both co-halves: each V row-piece gets 2x
                # the PE cover, absorbing the image-0 DMA trickle
                for c in range(4):
                    for h in range(2):
                        r0, hh = CHUNK_R0[c], CHUNK_H[c]
                        mv = hh * T  # moving dim: 448 or 224
                        pss = [
                            pspool.tile([128, 448], f32, name="ps") for _ in range(NK)
                        ]
                        for k in range(NK):
                            for dy in range(3):
                                nc.tensor.matmul(
                                    pss[k][:, 0:mv],
                                    lhsT=uslice(h, k, dy),
                                    rhs=vt3[:, k, (r0 + dy) * T : (r0 + dy + hh) * T],
                                    start=(dy == 0),
                                    stop=(dy == 2),
                                )
                        # output transform: ACT drains m1,m2; DVE combines.
                        # ye and yo share one staging tile so the chunk ships
                        # as a single DMA (halves the sync-ring issue count).
                        c1 = tpool.tile([128, 448], f16, name="tc1")
                        c2 = tpool.tile([128, 448], f16, name="tc2")
                        ta = tpool.tile([128, 448], f16, name="ta")
                        tb = tpool.tile([128, 448], f16, name="tb")
                        yy = opool.tile([128, 896], f16, name="yy")
                        nc.scalar.copy(out=c1[:, 0:mv], in_=pss[1][:, 0:mv])
                        nc.scalar.copy(out=c2[:, 0:mv], in_=pss[2][:, 0:mv])
                        nc.vector.tensor_tensor(
                            out=ta[:, 0:mv], in0=pss[0][:, 0:mv], in1=c1[:, 0:mv], op=ADD
                        )
                        nc.vector.tensor_tensor(
                            out=tb[:, 0:mv], in0=c1[:, 0:mv], in1=c2[:, 0:mv], op=SUB
                        )
                        # ye is a leaf (only consumed by its DMA): safe on the
                        # slower GpSimd without blocking the DVE FIFO -- but
                        # for the final two chunks use DVE so the kernel tail
                        # never waits on GpSimd
                        ye_engine = nc.vector if (n == IMGS - 1 and c == 3) else nc.gpsimd
                        ye_engine.tensor_tensor(
                            out=yy[:, 0:mv], in0=ta[:, 0:mv], in1=c2[:, 0:mv], op=ADD
                        )
                        nc.vector.tensor_tensor(
                            out=yy[:, 448 : 448 + mv],
                            in0=tb[:, 0:mv],
                            in1=pss[3][:, 0:mv],
                            op=SUB,
                        )
                        # single out-DMA per chunk on the sync ring: a wait on
                        # the slow GpSimd ye there only delays future V
                        # prefetch (slack-rich), never the transform chain
                        if mv == 448:
                            nc.sync.dma_start(
                                out=out[n, h, :, 2 * c * 448 : (2 * c + 2) * 448],
                                in_=yy[:, 0:896],
                            )
                        else:
                            out4 = out.rearrange("n h p (q e) -> n h p q e", e=448)
                            nc.sync.dma_start(
                                out=out4[n, h, :, 2 * c : 2 * c + 2, 0:mv],
                                in_=yy.rearrange("p (q e) -> p q e", e=448)[:, :, 0:mv],
                            )
                    # hoist next image's V DMA issues to mid-image so the
                    # transfers complete before that image starts
                    if c == 1 and n + 1 < IMGS:
                        vt_next = vpool.tile([CIN, NK * PLANE], f16)
                        vt3_next = vt_next.rearrange("p (k e) -> p k e", k=NK)
                        emit_v_dmas(n + 1, vt3_next)
                if n + 1 < IMGS:
                    vt3 = vt3_next

    _split_sync_waits(nc, mybir)
    return nc


def _prep_inputs(input_batch, weights):
    xf = np.asarray(input_batch, dtype=np.float32)
    xp = np.zeros((N_FULL, CIN, HP, HP), dtype=np.float32)
    xp[:, :, 1:-1, 1:-1] = xf
    e = xp[..., 0:56:2]
    o = xp[..., 1:57:2]
    e2 = xp[..., 2:58:2]
    o3 = xp[..., 3:58:2]
    V = np.stack([e - e2, o + e2, e2 - o, o - o3], axis=2).astype(np.float16)
    V = np.ascontiguousarray(V.reshape(N_FULL, CIN, NK * PLANE))

    wf = np.asarray(weights, dtype=np.float32)
    U = np.empty((3, NK, COUT, CIN), np.float32)
    for dy in range(3):
        g0, g1, g2 = wf[:, :, dy, 0], wf[:, :, dy, 1], wf[:, :, dy, 2]
        U[dy, 0] = g0
        U[dy, 1] = (g0 + g1 + g2) * 0.5
        U[dy, 2] = (g0 - g1 + g2) * 0.5
        U[dy, 3] = g2
    # w[ci, ((h*4 + k)*3 + dy)*128 + c] = U[dy, k, h*128 + c, ci]
    wt = np.ascontiguousarray(
        U.reshape(3, NK, 2, 128, CIN)
        .transpose(4, 2, 1, 0, 3)  # [ci, h, k, dy, c]
        .reshape(CIN, 2 * NK * 3 * 128)
        .astype(np.float16)
    )
    in_maps = []
    for i in range(N_CORES):
        in_maps.append(
            {"x": np.ascontiguousarray(V[i * IMGS : (i + 1) * IMGS]), "w": wt}
        )
    return in_maps


def _assemble(outs):
    # outs: list of [IMGS, 2, 128, 8*448] fp16 per core; layout
    # [n, half, co, chunk, phase, 448(padded)]
    full = np.concatenate(outs, axis=0).reshape(N_FULL, 2, 128, 4, 2, 448)
    res = np.empty((N_FULL, 2, 128, 2, H, T), np.float32)
    for c in range(4):
        r0, hh = CHUNK_R0[c], CHUNK_H[c]
        res[:, :, :, :, r0 : r0 + hh, :] = (
            full[:, :, :, c, :, 0 : hh * T]
            .astype(np.float32)
            .reshape(N_FULL, 2, 128, 2, hh, T)
        )
    res = res.transpose(0, 1, 2, 4, 5, 3)
    return np.ascontiguousarray(res.reshape(N_FULL, COUT, H, W))


def _run(input_batch, weights, trace=False):
    from concourse.bass_utils import run_bass_kernel_spmd

    if "nc" not in _CACHE:
        _CACHE["nc"] = _build()
    nc = _CACHE["nc"]
    in_maps = _prep_inputs(np.asarray(input_batch), np.asarray(weights))
    res = run_bass_kernel_spmd(nc, in_maps, list(range(N_CORES)), trace=trace)
    outs = [res.results[i]["out"] for i in range(N_CORES)]
    return _assemble(outs), res


def kernel(input_batch, weights):
    full, _ = _run(input_batch, weights, trace=False)
    return full


# revision 25
# speedup vs baseline: 1.0115x; 1.0115x over previous
"""Conv2D 3x3 (NCHW, OIHW, stride 1, pad 1) on 8 Trainium2 NeuronCores.

Problem shape: input (32, 128, 56, 56) fp32, weights (256, 128, 3, 3) fp32,
output (32, 256, 56, 56) fp32.

Strategy — 1D Winograd F(2,3) along the width axis:
  - Data-parallel over batch: 4 images per core, weights replicated.
  - Host precomputes the Winograd input transform: for each padded row and
    2-wide output tile t, the 4 transform planes
        v0 = d[2t]   - d[2t+2]
        v1 = d[2t+1] + d[2t+2]
        v2 = d[2t+2] - d[2t+1]
        v3 = d[2t+1] - d[2t+3]
    giving V[ci, k, 58 rows, 28 tiles] (fp16). Weights become
    U[dy,k][ci,co] = rows of G @ g (fp16).
  - Device: per (image, 16-row chunk, co-half) accumulate in PSUM
        m_k = sum_dy U[dy,k].T @ V[k][rows+dy]     (4 planes, 12 matmuls)
    with moving dim 448 = 16 rows x 28 tiles. This is 2/3 of the direct
    conv's MAC count (12x448 vs 9x(2x448) per 16 output rows), and the
    per-matmul LDWEIGHTS (~97ns) hides under the 448-cycle matmuls.
  - Output transform split across engines so the PE stays the bottleneck
    (DVE alone cannot drain 4 PSUM planes per chunk at 1 elem/cycle fp32):
        ACT:    c1 = m1, c2 = m2 (PSUM -> SBUF fp16 copies)
        DVE:    ta = m0 + c1, tb = c1 - c2 (2x fp16), y_odd = tb - m3
        GpSimd: y_even = ta + c2  (leaf op: nothing ever waits on GpSimd
                except its own chunk DMA; final chunks use DVE instead)
    y_even/y_odd ship as one merged DMA per chunk into a padded DRAM
    layout; the host interleaves even/odd columns and casts to fp32.
  - Scheduling notes (hard-won): out-DMAs must NOT share a sequencer with
    the ACT copies (a DMA waiting on GpSimd blocks the FIFO behind it and
    stalls the PSUM-release chain -> PE gaps). Boot-time DMA bandwidth is
    scarce: image 0's V planes and the weight pieces are split fine-grained
    across both HWDGE rings in first-use order, and 10 warmup matmuls
    bridge sequencer boot until the first operands land.
"""

import sys

sys.path.insert(0, "/opt/trn_rl_repo")

import numpy as np

N_CORES = 8
N_FULL = 32
IMGS = N_FULL // N_CORES  # images per core
CIN = 128
COUT = 256
H = W = 56
HP = 58  # padded rows
T = 28  # winograd tiles per row
PLANE = HP * T  # 1624 elements per transform plane
NK = 4  # transform planes
OUTP = H * T  # 1568 output elements per phase (even/odd)

# row chunks per (image, half): 16+16+16+8 output rows
CHUNK_R0 = (0, 16, 32, 48)
CHUNK_H = (16, 16, 16, 8)

_CACHE = {}


def _split_sync_waits(nc, mybir, max_waits=1):
    """The walrus build in this container rejects instructions carrying
    more than one semaphore wait; hoist extras onto preceding NOPs on the
    same engine (engine executes them in order, semantics preserved)."""
    ctr = 0
    for f in nc.m.functions:
        for bb in f.blocks:
            new_insts = []
            for ins in bb.instructions:
                si = getattr(ins, "sync_info", None)
                if si is not None and si.on_wait and len(si.on_wait) > max_waits:
                    waits = list(si.on_wait)
                    extra, keep = waits[:-max_waits], waits[-max_waits:]
                    for i in range(0, len(extra), max_waits):
                        ctr += 1
                        nop = mybir.InstNoOp(
                            name=f"{ins.name}_wsplit{ctr}",
                            engine=ins.engine,
                            sync_info=mybir.SyncInfo(
                                on_wait=extra[i : i + max_waits], on_update=[]
                            ),
                            bass_nofuse=True,
                        )
                        new_insts.append(nop)
                    si.on_wait = keep
                new_insts.append(ins)
            bb.instructions[:] = new_insts
    return ctr


def _build():
    import concourse.bass as bass
    import concourse.mybir as mybir
    import concourse.tile as tile

    f32 = mybir.dt.float32
    f16 = mybir.dt.float16
    ADD = mybir.AluOpType.add
    SUB = mybir.AluOpType.subtract

    nc = bass.Bass()
    x = nc.declare_dram_parameter("x", [IMGS, CIN, NK * PLANE], f16, isOutput=False)
    w = nc.declare_dram_parameter("w", [CIN, 2 * NK * 3 * 128], f16, isOutput=False)
    # out[n, half, co, chunk*2+phase, 448 (padded)]; phase 0 = even cols
    out = nc.declare_dram_parameter("out", [IMGS, 2, 128, 8 * 448], f16, isOutput=True)

    x3 = x.rearrange("n p (k e) -> n p k e", k=NK)

    with tile.TileContext(nc) as tc:
        with (
            tc.tile_pool(name="wpool", bufs=1) as wpool,
            tc.tile_pool(name="vpool", bufs=2) as vpool,
            tc.tile_pool(name="tpool", bufs=3) as tpool,
            tc.tile_pool(name="opool", bufs=4) as opool,
            tc.tile_pool(name="psum", bufs=8, space="PSUM") as pspool,
        ):
            # Short PE warmup (~1.7us) bridges the gap until the first
            # input DMA lands; the first real matmuls then run inside the
            # remaining HAM cold window (1.2 GHz until ~3.4us of PE busy).
            warm = wpool.tile([128, 256], f16, name="warm")
            nc.vector.memzero(warm[:])
            wps = pspool.tile([128, 448], f32, name="ps")
            for _ in range(10):
                nc.tensor.matmul(
                    wps[:, 0:256], lhsT=warm[:, 0:128], rhs=warm[:], start=True, stop=True
                )

            # wt pieces are interleaved with image-0's input pieces below so
            # each lands just before its first use: h0/k0 weights first, the
            # rest of h0 on the sync ring behind the k0/k2 input heads, h1
            # weights later on the scalar ring.
            wt = wpool.tile([CIN, 2 * NK * 3 * 128], f16)
            nc.scalar.dma_start(out=wt[:, 0:384], in_=w[:, 0:384])

            def uslice(h, k, dy):
                c0 = ((h * NK + k) * 3 + dy) * 128
                return wt[:, c0 : c0 + 128]

            def emit_v_dmas(n, vt3):
                if n == 0:
                    # fine-grained pieces: plane k's head rows (18 rows,
                    # 129KB) land just before that plane's first (cold)
                    # matmuls. Alternate rings so transfers run on disjoint
                    # DMA queue sets in parallel.
                    # interleave input-plane heads with the weight pieces in
                    # first-use order across both rings (boot-time DMA
                    # bandwidth is scarce; every byte before its consumer)
                    nc.sync.dma_start(out=vt3[:, 0, 0 : 18 * T], in_=x3[n, :, 0, 0 : 18 * T])
                    nc.scalar.dma_start(out=vt3[:, 1, 0 : 18 * T], in_=x3[n, :, 1, 0 : 18 * T])
                    nc.sync.dma_start(out=wt[:, 384:768], in_=w[:, 384:768])
                    nc.sync.dma_start(out=wt[:, 768:1152], in_=w[:, 768:1152])
                    nc.scalar.dma_start(out=vt3[:, 2, 0 : 18 * T], in_=x3[n, :, 2, 0 : 18 * T])
                    nc.sync.dma_start(out=vt3[:, 3, 0 : 18 * T], in_=x3[n, :, 3, 0 : 18 * T])
                    nc.scalar.dma_start(out=wt[:, 1152:1536], in_=w[:, 1152:1536])
                    nc.sync.dma_start(out=wt[:, 1536:1920], in_=w[:, 1536:1920])
                    nc.scalar.dma_start(out=wt[:, 1920:3072], in_=w[:, 1920:3072])
                    nc.sync.dma_start(
                        out=vt3[:, :, 18 * T : 34 * T], in_=x3[n, :, :, 18 * T : 34 * T]
                    )
                    nc.scalar.dma_start(
                        out=vt3[:, :, 34 * T : 50 * T], in_=x3[n, :, :, 34 * T : 50 * T]
                    )
                    nc.scalar.dma_start(
                        out=vt3[:, :, 50 * T : PLANE], in_=x3[n, :, :, 50 * T : PLANE]
                    )
                else:
                    # later images prefetch on the scalar ring: the sync
                    # ring stays dedicated to out-DMAs
                    nc.scalar.dma_start(
                        out=vt3[:, :, 0 : 18 * T], in_=x3[n, :, :, 0 : 18 * T]
                    )
                    nc.scalar.dma_start(
                        out=vt3[:, :, 18 * T : 35 * T], in_=x3[n, :, :, 18 * T : 35 * T]
                    )
                    nc.scalar.dma_start(
                        out=vt3[:, :, 35 * T : PLANE], in_=x3[n, :, :, 35 * T : PLANE]
                    )

            vt = vpool.tile([CIN, NK * PLANE], f16)
            vt3 = vt.rearrange("p (k e) -> p k e", k=NK)
            emit_v_dmas(0, vt3)

            for n in range(IMGS):
                # chunk-major over both co-halves: each V row-piece gets 2x
                # the PE cover, absorbing the image-0 DMA trickle
                for c in range(4):
                    for h in range(2):
                        r0, hh = CHUNK_R0[c], CHUNK_H[c]
                        mv = hh * T  # moving dim: 448 or 224
                        pss = [
                            pspool.tile([128, 448], f32, name="ps") for _ in range(NK)
                        ]
                        for k in range(NK):
                            for dy in range(3):
                                nc.tensor.matmul(
                                    pss[k][:, 0:mv],
                                    lhsT=uslice(h, k, dy),
                                    rhs=vt3[:, k, (r0 + dy) * T : (r0 + dy + hh) * T],
                                    start=(dy == 0),
                                    stop=(dy == 2),
                                )
                        # output transform: ACT drains m1,m2; DVE combines.
                        # ye and yo share one staging tile so the chunk ships
                        # as a single DMA (halves the sync-ring issue count).
                        c1 = tpool.tile([128, 448], f16, name="tc1")
                        c2 = tpool.tile([128, 448], f16, name="tc2")
                        ta = tpool.tile([128, 448], f16, name="ta")
                        tb = tpool.tile([128, 448], f16, name="tb")
                        yy = opool.tile([128, 896], f16, name="yy")
                        nc.scalar.copy(out=c1[:, 0:mv], in_=pss[1][:, 0:mv])
                        nc.scalar.copy(out=c2[:, 0:mv], in_=pss[2][:, 0:mv])
                        nc.vector.tensor_tensor(
                            out=ta[:, 0:mv], in0=pss[0][:, 0:mv], in1=c1[:, 0:mv], op=ADD
                        )
                        nc.vector.tensor_tensor(
                            out=tb[:, 0:mv], in0=c1[:, 0:mv], in1=c2[:, 0:mv], op=SUB
                        )
                        # ye is a leaf (only consumed by its DMA): safe on the
                        # slower GpSimd without blocking the DVE FIFO -- but
                        # for the final two chunks use DVE so the kernel tail
                        # never waits on GpSimd
                        ye_engine = nc.vector if (n == IMGS - 1 and c == 3) else nc.gpsimd
                        ye_engine.tensor_tensor(
                            out=yy[:, 0:mv], in0=ta[:, 0:mv], in1=c2[:, 0:mv], op=ADD
                        )
                        nc.vector.tensor_tensor(
                            out=yy[:, 448 : 448 + mv],
                            in0=tb[:, 0:mv],
                            in1=pss[3][:, 0:mv],
                            op=SUB,
                        )
                        # single out-DMA per chunk on the sync ring: a wait on
                        # the slow GpSimd ye there only delays future V
                        # prefetch (slack-rich), never the transform chain
                        if mv == 448:
                            nc.sync.dma_start(
                                out=out[n, h, :, 2 * c * 448 : (2 * c + 2) * 448],
                                in_=yy[:, 0:896],
                            )
                        else:
                            out4 = out.rearrange("n h p (q e) -> n h p q e", e=448)
                            nc.sync.dma_start(
                                out=out4[n, h, :, 2 * c : 2 * c + 2, 0:mv],
                                in_=yy.rearrange("p (q e) -> p q e", e=448)[:, :, 0:mv],
                            )
                    # hoist next image's V DMA issues to mid-image so the
                    # transfers complete before that image starts
                    if c == 1 and n + 1 < IMGS:
                        vt_next = vpool.tile([CIN, NK * PLANE], f16)
                        vt3_next = vt_next.rearrange("p (k e) -> p k e", k=NK)
                        emit_v_dmas(n + 1, vt3_next)
                if n + 1 < IMGS:
                    vt3 = vt3_next

    _split_sync_waits(nc, mybir)
    return nc


def _prep_inputs(input_batch, weights):
    xf = np.asarray(input_batch, dtype=np.float32)
    xp = np.zeros((N_FULL, CIN, HP, HP), dtype=np.float32)
    xp[:, :, 1:-1, 1:-1] = xf
    e = xp[..., 0:56:2]
    o = xp[..., 1:57:2]
    e2 = xp[..., 2:58:2]
    o3 = xp[..., 3:58:2]
    V = np.stack([e - e2, o + e2, e2 - o, o - o3], axis=2).astype(np.float16)
    V = np.ascontiguousarray(V.reshape(N_FULL, CIN, NK * PLANE))

    wf = np.asarray(weights, dtype=np.float32)
    U = np.empty((3, NK, COUT, CIN), np.float32)
    for dy in range(3):
        g0, g1, g2 = wf[:, :, dy, 0], wf[:, :, dy, 1], wf[:, :, dy, 2]
        U[dy, 0] = g0
        U[dy, 1] = (g0 + g1 + g2) * 0.5
        U[dy, 2] = (g0 - g1 + g2) * 0.5
        U[dy, 3] = g2
    # w[ci, ((h*4 + k)*3 + dy)*128 + c] = U[dy, k, h*128 + c, ci]
    wt = np.ascontiguousarray(
        U.reshape(3, NK, 2, 128, CIN)
        .transpose(4, 2, 1, 0, 3)  # [ci, h, k, dy, c]
        .reshape(CIN, 2 * NK * 3 * 128)
        .astype(np.float16)
    )
    in_maps = []
    for i in range(N_CORES):
        in_maps.append(
            {"x": np.ascontiguousarray(V[i * IMGS : (i + 1) * IMGS]), "w": wt}
        )
    return in_maps


def _assemble(outs):
    # outs: list of [IMGS, 2, 128, 8*448] fp16 per core; layout
    # [n, half, co, chunk, phase, 448(padded)]
    full = np.concatenate(outs, axis=0).reshape(N_FULL, 2, 128, 4, 2, 448)
    res = np.empty((N_FULL, 2, 128, 2, H, T), np.float32)
    for c in range(4):
        r0, hh = CHUNK_R0[c], CHUNK_H[c]
        res[:, :, :, :, r0 : r0 + hh, :] = (
            full[:, :, :, c, :, 0 : hh * T]
            .astype(np.float32)
            .reshape(N_FULL, 2, 128, 2, hh, T)
        )
    res = res.transpose(0, 1, 2, 4, 5, 3)
    return np.ascontiguousarray(res.reshape(N_FULL, COUT, H, W))


def _run(input_batch, weights, trace=False):
    from concourse.bass_utils import run_bass_kernel_spmd

    if "nc" not in _CACHE:
        _CACHE["nc"] = _build()
    nc = _CACHE["nc"]
    in_maps = _prep_inputs(np.asarray(input_batch), np.asarray(weights))
    res = run_bass_kernel_spmd(nc, in_maps, list(range(N_CORES)), trace=trace)
    outs = [res.results[i]["out"] for i in range(N_CORES)]
    return _assemble(outs), res


def kernel(input_batch, weights):
    full, _ = _run(input_batch, weights, trace=False)
    return full
